# revision 18
# baseline (speedup 1.0000x reference)
"""Trainium2 Bass kernel for GQA attention (B=4, S=2048, HID=896, H=14, KV=2, D=64).

Sharding: 8 cores = 4 batches x 2 KV-head groups. Core c handles batch c//2,
query heads [g*7, (g+1)*7) with g = c%2 (exactly one KV head per core). Each
core computes its 448-channel slice of attn output and the partial projection
y_g = ao_g @ Wo[g*448:(g+1)*448, :]; the host sums the two partials per batch.

The kernel is ACT(ScalarE)-bound: softmax exp over the causal score area is
~122K columns/core at ~0.83ns/col + ~0.3us/call. Design follows from that:

  A) Projections + RoPE + PE-transposes, emitted per 4-s-tile chunk with
     chains pipelined across psum tags. All of A runs before B: PE's lead
     over ACT is capped by the 2 scores psum slots (~3us), so interleaving
     A's PE work into B always stalls the bottleneck ACT stream.
  B) Per (head, qc-chunk-of-512) unit: scoresT[k,q] = kT.T @ qT in triples
     of 3 k-tiles sharing one [128,1536] psum (3 banks); ONE exp call per
     triple spanning diagonal gaps (gap cols hold stale-but-bounded scores;
     their exp is finite and never consumed). Structural causality: only
     k-tiles <= diagonal computed; diagonal 128x128 blocks masked by a
     triangular 0/1 multiply after exp. A single flat software pipeline runs
     across ALL (qc, head, group) so ACT never drains at unit boundaries.
     Every 3rd full (non-diagonal) triple's exp is offloaded to the DVE as a
     Schraudolph fp16 fast-exp (tensor_scalar mul+add -> round-to-u16 ->
     bitcast fp16, ~3% elementwise, ~3e-3 end-to-end); this trades idle DVE
     time for critical ACT time. attn@[v|1] accumulates per unit into PSUM
     [65,512]; row 64 = softmax denominators.
  C) Normalization: one batched Ln+Exp(-x) reciprocal over the packed
     denominator tiles (2 ACT table switches total), per-head DMA-broadcast
     via a DRAM roundtrip, then y = ao @ Wo_g with PSUM evacuation
     alternating between DVE and ACT.

PSUM (8 banks): "sp" 2x[128,1536] (6) shared by scores triples, phase-C
y_ps, and phase-A q/transpose staging; "o" 2x[65,512] (2) shared by attn@v
accumulators and phase-A kv/k-transpose staging.

The causal mask input is never loaded: exp(-1e9 + s) == 0.0 exactly in fp32.
mm_dt=f16 (default): PE streams 1 cycle/row; rel err ~3.3e-3 incl. fast-exp.
"""
import math
import os
import numpy as np

import concourse.bass as bass
import concourse.mybir as mybir
import concourse.tile as tile
from concourse import bacc
from concourse.masks import make_identity

F32 = mybir.dt.float32
F32R = mybir.dt.float32r
F16 = mybir.dt.float16
U16 = mybir.dt.uint16
AF = mybir.ActivationFunctionType
ALU = mybir.AluOpType
# fp16 Schraudolph fast-exp: bits = round(x*EXPA + EXPB) viewed as fp16.
# |rel err| <= ~3%; applied to a ~1/3 subset of full k-tile groups the
# end-to-end error contribution is ~3e-3 (gate is 2e-2).
EXPA = 1477.3197218702985      # 1024/ln(2)
EXPB = 15293.44                # (15 - 0.0650) * 1024

B, S, HID = 4, 2048, 896
H, KV, D = 14, 2, 64
HL = H // KV          # 7 local query heads per core
GD = HL * D           # 448 local channels
KCH = HID // 128      # 7 contraction chunks
N_CORES = 8


def _bc7(ap_small):
    """[128, 32] cos/sin slice -> broadcast over the 7 heads: [128, 7, 32]."""
    return bass.AP(
        tensor=ap_small.tensor,
        offset=ap_small.offset,
        ap=[list(ap_small.ap[0]), [0, HL], list(ap_small.ap[1])],
    )


def build(s=S, mm_dt=None, reps=1, phases="A2BC"):
    if mm_dt is None:
        mm_dt = MM_DT
    ST = s // 128           # s-tiles
    QC = s // 512           # q chunks
    nc = bacc.Bacc("TRN2", target_bir_lowering=False, debug=False,
                   num_devices=N_CORES)

    xT = nc.dram_tensor("xT", [HID, s], mm_dt, kind="ExternalInput").ap()
    wq = nc.dram_tensor("wq", [HID, GD], mm_dt, kind="ExternalInput").ap()
    wkv = nc.dram_tensor("wkv", [HID, 128], mm_dt, kind="ExternalInput").ap()
    wo = nc.dram_tensor("wo", [GD, HID], mm_dt, kind="ExternalInput").ap()
    cosr = nc.dram_tensor("cosr", [128, ST, 32], F32, kind="ExternalInput").ap()
    sinr = nc.dram_tensor("sinr", [128, ST, 32], F32, kind="ExternalInput").ap()
    triu = nc.dram_tensor("triu", [128, 128], mm_dt, kind="ExternalInput").ap()
    y = nc.dram_tensor("y", [s, HID], F32, kind="ExternalOutput").ap()
    rec_dt0 = F16 if mm_dt == F16 else F32
    dram_rec = nc.dram_tensor("dram_rec", [HL, s], rec_dt0).ap()

    with tile.TileContext(nc) as tc:
        with (
            tc.tile_pool(name="wp", bufs=1) as wp,
            tc.tile_pool(name="per", bufs=1) as per,
            tc.tile_pool(name="tmp", bufs=2) as tmp,
        ):
            def _body():
                # ---- small loads (weights/tables stay resident) ----
                wq_sb = wp.tile([128, KCH, GD], mm_dt, tag="wq", name="wq")
                nc.sync.dma_start(out=wq_sb[:], in_=wq.rearrange("(k p) m -> p k m", p=128))
                wkv_sb = wp.tile([128, KCH, 128], mm_dt, tag="wkv", name="wkv")
                nc.sync.dma_start(out=wkv_sb[:], in_=wkv.rearrange("(k p) m -> p k m", p=128))
                wo_sb = wp.tile([128, 4, HID], mm_dt, tag="wo", name="wo")
                for cc in range(4):
                    w = 128 if cc < 3 else 64
                    nc.sync.dma_start(out=wo_sb[0:w, cc, :], in_=wo[cc * 128:cc * 128 + w, :])
                cos_sb = wp.tile([128, ST, 32], F32, tag="cos", name="cos")
                nc.sync.dma_start(out=cos_sb[:], in_=cosr)
                sin_sb = wp.tile([128, ST, 32], F32, tag="sin", name="sin")
                nc.sync.dma_start(out=sin_sb[:], in_=sinr)
                triu_sb = wp.tile([128, 128], mm_dt, tag="triu", name="triu")
                nc.sync.dma_start(out=triu_sb[:], in_=triu)
                idn = wp.tile([128, 128], F32, tag="idn", name="idn")
                make_identity(nc, idn[:])
                idn_r = wp.tile([128, 128], mm_dt, tag="idnr", name="idnr")
                nc.vector.tensor_copy(idn_r[:], idn[:])
                idn_mm = idn_r[:]
                # PE warmup: ~4us of dummy matmuls while the input DMAs land,
                # so the HAM clock-gate reaches 8/8 before real work starts
                with tc.tile_pool(name="psW", bufs=1, space="PSUM") as psW:
                    wps = psW.tile([128, 128], F32, tag="w", name="w")
                    for _ in range(72):
                        nc.tensor.matmul(wps[:], idn_mm, idn_mm,
                                         start=True, stop=True)

                # ---- persistent intermediates ----
                # qT / aoT unified: [128, 4, s]; chunk j holds heads (2j, 2j+1),
                # head h lives at chunk h//2, partition half (h%2)*64
                q_all = per.tile([128, 4, s], mm_dt, tag="q_all", name="q_all")
                ao_all = per.tile([128, 4, s], mm_dt, tag="ao_all", name="ao_all")
                rec_dt = F16 if mm_dt == F16 else F32
                araw_all = per.tile([128, 4, s], mm_dt, tag="araw", name="araw")
                # denominator rows live at engine-legal base partitions
                # {0,32,64,96}: head h -> tile h//4, row 32*(h%4)
                den_t = [per.tile([128, s], F32, tag=f"dn{j}", name=f"dn{j}")
                         for j in range(2)]
                kT2 = per.tile([128, s], mm_dt, tag="kT2", name="kT2")
                k_all = per.tile([128, ST, 64], mm_dt, tag="k_all", name="k_all")
                v_all = per.tile([128, ST, 65], mm_dt, tag="v_all", name="v_all")
                if mm_dt == F16:
                    nc.vector.memset(
                        v_all[:, :, 64:65].bitcast(mybir.dt.uint16), 0x3C00)
                else:
                    nc.vector.memset(v_all[:, :, 64:65].bitcast(F32), 1.0)

                # ========== pipelined phases: A feeds B; C trails ==========
                # One PSUM pool, 8 banks total:
                #   tag "sp": [128,1536] F32 x2 bufs (6 banks) - scores
                #       triples (GK=3) + phase-C y_ps + A-phase staging
                #   tag "o":  [65,512] F32 x2 (2 banks) - attn@v accumulator
                #       + A-phase kv/transpose staging (pre-B only)
                GK = 3
                LOOK = 1
                with tc.tile_pool(name="xp", bufs=1) as xp, \
                     tc.tile_pool(name="ps", bufs=1, space="PSUM") as ps, \
                     tc.tile_pool(name="expp", bufs=1) as expp:
                    xT_sb = [xp.tile([128, s], mm_dt, tag=f"xT{k}", name=f"xT{k}")
                             for k in range(KCH)]
                    for k in range(KCH):
                        nc.sync.dma_start(out=xT_sb[k][:],
                                          in_=xT[k * 128:(k + 1) * 128, :])

                    # ---- A-chain emitters: 3 steps per s-tile ----
                    rope_pend = {}

                    PSB = {"sp": 2, "o": 2}

                    q_pend = {}

                    def a_st1a(st, tq="aq"):
                        # q projection first half
                        q_ps = ps.tile([128, GD], F32, tag=tq, name=tq,
                                       bufs=PSB[tq])
                        for kc in range(4):
                            nc.tensor.matmul(q_ps[:],
                                             xT_sb[kc][:, st * 128:(st + 1) * 128],
                                             wq_sb[:, kc, :],
                                             start=(kc == 0), stop=False)
                        q_pend[st] = q_ps

                    def a_st1(st, tq="aq"):
                        # q projection second half + RoPE (parked in rope_pend)
                        q_ps = q_pend.pop(st)
                        for kc in range(4, KCH):
                            nc.tensor.matmul(q_ps[:],
                                             xT_sb[kc][:, st * 128:(st + 1) * 128],
                                             wq_sb[:, kc, :],
                                             start=False, stop=(kc == KCH - 1))
                        qv = q_ps[:].rearrange("p (h u two) -> p h u two", two=2, u=32)
                        e, o = qv[:, :, :, 0], qv[:, :, :, 1]
                        cb = _bc7(cos_sb[:, st, :])
                        sb_ = _bc7(sin_sb[:, st, :])
                        t1 = tmp.tile([128, HL, 32], F32, tag="t1", name="t1", bufs=1)
                        t2 = tmp.tile([128, HL, 32], F32, tag="t2", name="t2", bufs=1)
                        t3 = tmp.tile([128, HL, 32], F32, tag="t3", name="t3", bufs=1)
                        t4 = tmp.tile([128, HL, 32], F32, tag="t4", name="t4", bufs=1)
                        nc.vector.tensor_mul(t1[:], e, cb)
                        nc.vector.tensor_mul(t2[:], o, sb_)
                        nc.vector.tensor_mul(t3[:], e, sb_)
                        nc.vector.tensor_mul(t4[:], o, cb)
                        q_rot = tmp.tile([128, GD], mm_dt, tag="qrot", name="qrot",
                                         bufs=2)
                        qrv = q_rot[:].rearrange("p (h u two) -> p h u two", two=2, u=32)
                        nc.gpsimd.tensor_sub(qrv[:, :, :, 0], t1[:], t2[:])
                        nc.gpsimd.tensor_add(qrv[:, :, :, 1], t3[:], t4[:])
                        rope_pend[st] = q_rot

                    def a_st2(st, tkv="aq"):
                        # k/v projection + staging
                        kv_ps = ps.tile([128, 128], F32, tag=tkv, name=tkv,
                                        bufs=PSB[tkv])
                        for kc in range(KCH):
                            nc.tensor.matmul(kv_ps[:],
                                             xT_sb[kc][:, st * 128:(st + 1) * 128],
                                             wkv_sb[:, kc, :],
                                             start=(kc == 0), stop=(kc == KCH - 1))
                        nc.vector.tensor_copy(k_all[:, st, :], kv_ps[:, 0:64])
                        nc.vector.tensor_copy(v_all[:, st, 0:64], kv_ps[:, 64:128])

                    def a_st3(st, ttr="aq"):
                        # PE-transpose q_rot to channel-major q_all
                        q_rot = rope_pend.pop(st)
                        t_ps = ps.tile([128, 4, 128], mm_dt, tag=ttr, name=ttr,
                                       bufs=PSB[ttr])
                        for cc in range(4):
                            w = 128 if cc < 3 else 64
                            nc.tensor.transpose(t_ps[0:w, cc, :],
                                                q_rot[:, cc * 128:cc * 128 + w],
                                                idn_mm)
                        nc.vector.tensor_copy(
                            q_all[:, 0:3, st * 128:(st + 1) * 128], t_ps[:, 0:3, :])
                        nc.vector.tensor_copy(
                            q_all[0:64, 3, st * 128:(st + 1) * 128], t_ps[0:64, 3, :])

                    def a_kstep(c, part=2):
                        # part 0: RoPE on k for chunk c; part 1: transpose+dup
                        if part == 2:
                            a_kstep(c, 0)
                            a_kstep(c, 1)
                            return
                        if part == 1:
                            for st in range(4 * c, 4 * c + 4):
                                t_ps = ps.tile([128, 128], mm_dt, tag="o",
                                               name="o", bufs=2)[0:64, :]
                                nc.tensor.transpose(t_ps, k_all[:, st, :], idn_mm)
                                nc.vector.tensor_copy(
                                    kT2[0:64, st * 128:(st + 1) * 128], t_ps)
                                nc.vector.tensor_copy(
                                    kT2[64:128, st * 128:(st + 1) * 128], t_ps)
                            return
                        kv4 = k_all[:, 4 * c:4 * c + 4, :].rearrange(
                            "p s (u two) -> p s u two", two=2)
                        ke, ko = kv4[:, :, :, 0], kv4[:, :, :, 1]
                        csl = cos_sb[:, 4 * c:4 * c + 4, :]
                        ssl = sin_sb[:, 4 * c:4 * c + 4, :]
                        k1 = tmp.tile([128, 4, 32], F32, tag="t1", name="t1", bufs=1)
                        k2 = tmp.tile([128, 4, 32], F32, tag="t2", name="t2", bufs=1)
                        k3 = tmp.tile([128, 4, 32], F32, tag="t3", name="t3", bufs=1)
                        k4 = tmp.tile([128, 4, 32], F32, tag="t4", name="t4", bufs=1)
                        nc.vector.tensor_mul(k1[:], ke, csl)
                        nc.vector.tensor_mul(k2[:], ko, ssl)
                        nc.vector.tensor_mul(k3[:], ke, ssl)
                        nc.vector.tensor_mul(k4[:], ko, csl)
                        nc.vector.tensor_sub(ke, k1[:], k2[:])
                        nc.vector.tensor_add(ko, k3[:], k4[:])

                    # ---- all of phase A up front (B's ACT stream must
                    # never wait on inserted PE work: PE lead over ACT is
                    # capped at the 2 "sp" slots, so interleaving A into B
                    # stalls the bottleneck engine) ----
                    for c in range(QC):
                        t0, t1, t2, t3 = 4 * c, 4 * c + 1, 4 * c + 2, 4 * c + 3
                        a_st1a(t0, "sp"); a_st1(t0, "sp")
                        a_st1a(t1, "sp"); a_st1(t1, "sp")
                        a_st2(t0, "o"); a_st2(t1, "o")
                        a_st1a(t2, "sp"); a_st1(t2, "sp")
                        a_st1a(t3, "sp"); a_st1(t3, "sp")
                        a_st3(t0, "o"); a_st3(t1, "o")
                        a_st2(t2, "sp"); a_st2(t3, "sp")
                        a_kstep(c, 0)
                        a_st3(t2, "o"); a_st3(t3, "o")
                        a_kstep(c, 1)

                    # ---- B: per-qc flat pipeline over (head, group) ----
                    fullctr = [0]

                    def emit_group(h, qc, gi):
                        half = (h % 2) * 64
                        jq = h // 2
                        nkt = 4 * (qc + 1)
                        s_ps = ps.tile([128, GK * 512], F32, tag="sp", name="sp",
                                       bufs=2)
                        ex = expp.tile([128, GK * 512], mm_dt, tag="ex",
                                       name="ex", bufs=3)
                        info = []
                        for j in range(GK):
                            kt = GK * gi + j
                            if kt >= nkt:
                                break
                            rrel = kt - 4 * qc
                            off = 128 * rrel if rrel >= 0 else 0
                            N = 512 - off
                            nc.tensor.matmul(
                                s_ps[:, 512 * j + off:512 * (j + 1)],
                                kT2[half:half + 64, kt * 128:(kt + 1) * 128],
                                q_all[half:half + 64, jq,
                                      qc * 512 + off:(qc + 1) * 512],
                                start=True, stop=True)
                            info.append((kt, 512 * j + off, off, N, rrel))
                        lo = info[0][1]
                        hi = info[-1][1] + info[-1][3]
                        full = info[-1][4] < 0 and len(info) == GK
                        if full and mm_dt == F16:
                            fullctr[0] += 1
                            if fullctr[0] % 2 == 0:
                                # DVE fast-exp offload: frees the ACT
                                # bottleneck at ~1.6x the DVE cost
                                ft = tmp.tile([128, GK * 512], F32, tag="fet",
                                              name="fet", bufs=2)
                                nc.vector.tensor_scalar(
                                    out=ft[:, lo:hi], in0=s_ps[:, lo:hi],
                                    scalar1=EXPA, scalar2=EXPB,
                                    op0=ALU.mult, op1=ALU.add)
                                nc.vector.tensor_copy(
                                    ex[:, lo:hi].bitcast(U16), ft[:, lo:hi])
                                return ex, info
                        nc.scalar.activation(out=ex[:, lo:hi],
                                             in_=s_ps[:, lo:hi], func=AF.Exp)
                        for kt, base, off, N, rrel in info:
                            if rrel >= 0:
                                nc.vector.tensor_mul(
                                    ex[:, base:base + 128],
                                    ex[:, base:base + 128], triu_sb[:])
                        return ex, info

                    lts = []
                    units = []
                    for qc in range(QC):
                        nkt = 4 * (qc + 1)
                        ngrp = (nkt + GK - 1) // GK
                        for h in range(HL):
                            units.append((h, qc, ngrp, nkt))
                    flat = [(ui, gi) for ui, u in enumerate(units)
                            for gi in range(u[2])]
                    emitted = {}
                    ptr = 0
                    o_ps = {}
                    for idx in range(len(flat)):
                        while ptr < len(flat) and ptr <= idx + LOOK:
                            ui, gi = flat[ptr]
                            h, qc, ngrp, nkt = units[ui]
                            emitted[(ui, gi)] = emit_group(h, qc, gi)
                            ptr += 1
                        ui, gi = flat[idx]
                        h, qc, ngrp, nkt = units[ui]
                        half = (h % 2) * 64
                        jq = h // 2
                        if gi == 0:
                            o_ps[ui] = ps.tile([65, 512], F32, tag="o",
                                               name="o", bufs=2)
                        ex, info = emitted.pop((ui, gi))
                        for kt, base, off, N, rrel in info:
                            nc.tensor.matmul(
                                o_ps[ui][:, off:512], v_all[:, kt, :],
                                ex[:, base:base + N],
                                start=(kt == 0), stop=(kt == nkt - 1))
                        if gi == ngrp - 1:
                            op = o_ps.pop(ui)
                            nc.vector.tensor_copy(
                                araw_all[half:half + 64, jq,
                                         qc * 512:(qc + 1) * 512],
                                op[0:64, :])
                            row = 32 * (h % 4)
                            nc.vector.tensor_copy(
                                den_t[h // 4][row:row + 1,
                                              qc * 512:(qc + 1) * 512],
                                op[64:65, :])

                    # ---- batched softmax normalization (all heads) ----
                    for j in range(2):
                        ltj = tmp.tile([128, s], F32, tag=f"lt{j}",
                                       name=f"lt{j}", bufs=1)
                        nc.scalar.activation(out=ltj[:], in_=den_t[j][:],
                                             func=AF.Ln)
                        lts.append(ltj)
                    rec2 = []
                    for j in range(2):
                        recj = tmp.tile([128, s], rec_dt, tag=f"rc{j}",
                                        name=f"rc{j}", bufs=1)
                        nc.scalar.activation(out=recj[:], in_=lts[j][:],
                                             func=AF.Exp, scale=-1.0)
                        rec2.append(recj)
                    for h in range(HL):
                        row = 32 * (h % 4)
                        nc.sync.dma_start(out=dram_rec[h:h + 1, :],
                                          in_=rec2[h // 4][row:row + 1, :])
                    for h in range(HL):
                        # broadcast via DRAM roundtrip (partition-broadcast
                        # reads are legal on DRAM-source DMAs and pipeline on
                        # the DMA engines)
                        half = (h % 2) * 64
                        rbF = tmp.tile([128, s], rec_dt, tag="rbF", name="rbF")
                        nc.sync.dma_start(out=rbF[:], in_=bass.AP(
                            tensor=dram_rec.tensor, offset=h * s,
                            ap=[[0, 128], [1, s]]))
                        for qc in range(QC):
                            nc.vector.tensor_mul(
                                ao_all[half:half + 64, h // 2,
                                       qc * 512:(qc + 1) * 512],
                                araw_all[half:half + 64, h // 2,
                                         qc * 512:(qc + 1) * 512],
                                rbF[half:half + 64, qc * 512:(qc + 1) * 512])

                    # ---- C: output projection ----
                    for st in range(ST):
                        y_ps = ps.tile([128, HID], F32, tag="sp", name="sp",
                                       bufs=2)
                        for cc in range(4):
                            w = 128 if cc < 3 else 64
                            lhsT = ao_all[0:w, cc, st * 128:(st + 1) * 128]
                            nc.tensor.matmul(y_ps[:, 0:512], lhsT,
                                             wo_sb[0:w, cc, 0:512],
                                             start=(cc == 0), stop=(cc == 3))
                            nc.tensor.matmul(y_ps[:, 512:896], lhsT,
                                             wo_sb[0:w, cc, 512:896],
                                             start=(cc == 0), stop=(cc == 3))
                        y_sb = tmp.tile([128, HID], F32, tag="ysb", name="ysb")
                        if st % 2 == 0:
                            nc.vector.tensor_copy(y_sb[:], y_ps[:, 0:896])
                        else:
                            nc.scalar.copy(out=y_sb[:], in_=y_ps[:, 0:896])
                        nc.sync.dma_start(out=y[st * 128:(st + 1) * 128, :],
                                          in_=y_sb[:])

            if reps > 1:
                with tc.For_i(0, reps, 1):
                    _body()
            else:
                _body()

    nc.compile()
    return nc


# ---------------------------------------------------------------------------
# host-side sharding + execution
# ---------------------------------------------------------------------------

def round_f32r(a):
    """Round fp32 array to fp32r (RNE to 11 mantissa bits) -- bit-exact match
    of the hardware's casting DMA, verified by SBUF readback."""
    b = np.ascontiguousarray(a, dtype=np.float32).view(np.uint32)
    lsb = (b >> np.uint32(12)) & np.uint32(1)
    r = ((b + np.uint32(0x7FF) + lsb) & np.uint32(0xFFFFF000))
    return r.view(np.float32)


MM_DT = {"f32r": F32R, "f16": F16, "f32": F32}[os.environ.get("MM_DT", "f16")]


def _cvt(a, mm_dt):
    if mm_dt == F16:
        return np.ascontiguousarray(np.asarray(a, dtype=np.float32)).astype(np.float16)
    if mm_dt == F32R:
        return round_f32r(a)
    return np.ascontiguousarray(a, dtype=np.float32)


def make_in_maps(x, freqs_cos, freqs_sin, Wq, Wk, Wv, Wo, s=S, mm_dt=None):
    if mm_dt is None:
        mm_dt = MM_DT
    ST = s // 128
    scale = 1.0 / math.sqrt(D)
    cosr = np.ascontiguousarray(
        np.asarray(freqs_cos).reshape(ST, 128, 32).transpose(1, 0, 2)).astype(np.float32)
    sinr = np.ascontiguousarray(
        np.asarray(freqs_sin).reshape(ST, 128, 32).transpose(1, 0, 2)).astype(np.float32)
    triu = _cvt(np.triu(np.ones((128, 128), dtype=np.float32)), mm_dt)
    in_maps = []
    for c in range(N_CORES):
        b, g = c // 2, c % 2
        in_maps.append({
            "xT": _cvt(np.asarray(x)[b].T, mm_dt),
            "wq": _cvt(np.asarray(Wq)[:, g * GD:(g + 1) * GD] * scale, mm_dt),
            "wkv": _cvt(np.concatenate(
                [np.asarray(Wk)[:, g * D:(g + 1) * D],
                 np.asarray(Wv)[:, g * D:(g + 1) * D]], axis=1), mm_dt),
            "wo": _cvt(np.asarray(Wo)[g * GD:(g + 1) * GD, :], mm_dt),
            "cosr": cosr, "sinr": sinr, "triu": triu,
        })
    return in_maps


_RUNNER = None


class _Runner:
    """Minimal SPMD executor over axon PJRT (self-contained copy)."""

    def __init__(self, nc, n_cores):
        import jax
        from jax.sharding import Mesh, PartitionSpec, NamedSharding
        from jax.experimental.shard_map import shard_map
        from concourse.bass2jax import (_bass_exec_p, install_neuronx_cc_hook,
                                        partition_id_tensor)
        install_neuronx_cc_hook()
        self.jax = jax
        self.n_cores = n_cores
        partition_name = (nc.partition_id_tensor.name
                          if nc.partition_id_tensor else None)
        in_names, out_names, out_avals = [], [], []
        for alloc in nc.m.functions[0].allocations:
            if not isinstance(alloc, mybir.MemoryLocationSet):
                continue
            name = alloc.memorylocations[0].name
            if alloc.kind == "ExternalInput":
                if name != partition_name:
                    in_names.append(name)
            elif alloc.kind == "ExternalOutput":
                out_names.append(name)
                out_avals.append(jax.core.ShapedArray(
                    tuple(alloc.tensor_shape), mybir.dt.np(alloc.dtype)))
        self.in_names, self.out_names, self.out_avals = in_names, out_names, out_avals
        n_params, n_outs = len(in_names), len(out_avals)
        all_names = in_names + out_names
        if partition_name is not None:
            all_names.append(partition_name)

        def _body(*args):
            operands = list(args)
            if partition_name is not None:
                operands.append(partition_id_tensor())
            return tuple(_bass_exec_p.bind(
                *operands, out_avals=tuple(out_avals), in_names=tuple(all_names),
                out_names=tuple(out_names), lowering_input_output_aliases=(),
                sim_require_finite=False, sim_require_nnan=False, nc=nc))

        devices = jax.devices()[:n_cores]
        self.mesh = Mesh(np.asarray(devices), ("core",))
        self.sharding = NamedSharding(self.mesh, PartitionSpec("core"))
        in_specs = (PartitionSpec("core"),) * (n_params + n_outs)
        out_specs = (PartitionSpec("core"),) * n_outs
        self.fn = jax.jit(
            shard_map(_body, mesh=self.mesh, in_specs=in_specs,
                      out_specs=out_specs, check_rep=False),
            donate_argnums=tuple(range(n_params, n_params + n_outs)),
            keep_unused=True)
        zshapes = [(n_cores * a.shape[0], *a.shape[1:]) for a in out_avals]
        zdtypes = [a.dtype for a in out_avals]
        self.make_zeros = jax.jit(
            lambda: tuple(jax.numpy.zeros(sh, dt)
                          for sh, dt in zip(zshapes, zdtypes)),
            out_shardings=tuple(self.sharding for _ in zshapes))

    def prep(self, in_maps):
        return [self.jax.device_put(
            np.concatenate([np.asarray(in_maps[c][n]) for c in range(self.n_cores)],
                           axis=0), self.sharding)
            for n in self.in_names]

    def run(self, dev_in):
        return self.fn(*dev_in, *self.make_zeros())

    def split(self, outs):
        res = []
        for c in range(self.n_cores):
            res.append({n: np.asarray(outs[i]).reshape(
                self.n_cores, *self.out_avals[i].shape)[c]
                for i, n in enumerate(self.out_names)})
        return res


def get_runner():
    global _RUNNER
    if _RUNNER is None:
        _RUNNER = _Runner(build(), N_CORES)
    return _RUNNER


def kernel(x, freqs_cos, freqs_sin, mask, Wq, Wk, Wv, Wo):
    x = np.asarray(x, dtype=np.float32)
    in_maps = make_in_maps(np.asarray(x), np.asarray(freqs_cos),
                           np.asarray(freqs_sin), np.asarray(Wq),
                           np.asarray(Wk), np.asarray(Wv), np.asarray(Wo))
    r = get_runner()
    outs = r.run(r.prep(in_maps))
    res = r.split(outs)
    out = np.empty((B, S, HID), dtype=np.float32)
    for b in range(B):
        out[b] = res[2 * b]["y"] + res[2 * b + 1]["y"]
    return out



# revision 19
# speedup vs baseline: 1.1554x; 1.1554x over previous
"""Trainium2 Bass kernel for GQA attention (B=4, S=2048, HID=896, H=14, KV=2, D=64).

Sharding: 8 cores = 4 batches x 2 KV-head groups. Core c handles batch c//2,
query heads [g*7, (g+1)*7) with g = c%2 (exactly one KV head per core). Each
core computes its 448-channel slice of attn output and the partial projection
y_g = ao_g @ Wo[g*448:(g+1)*448, :]; the host sums the two partials per batch.

The kernel is ACT(ScalarE)-bound: softmax exp over the causal score area is
~122K columns/core at ~0.83ns/col + ~0.3us/call. Design follows from that:

  A) Projections + RoPE + PE-transposes, emitted per 4-s-tile chunk with
     chains pipelined across psum tags. All of A runs before B: PE's lead
     over ACT is capped by the 2 scores psum slots (~3us), so interleaving
     A's PE work into B always stalls the bottleneck ACT stream.
  B) Per (head, qc-chunk-of-512) unit: scoresT[k,q] = kT.T @ qT in triples
     of 3 k-tiles sharing one [128,1536] psum (3 banks); ONE exp call per
     triple spanning diagonal gaps (gap cols hold stale-but-bounded scores;
     their exp is finite and never consumed). Structural causality: only
     k-tiles <= diagonal computed; diagonal 128x128 blocks masked by a
     triangular 0/1 multiply after exp. A single flat software pipeline runs
     across ALL (qc, head, group) so ACT never drains at unit boundaries.
     Every 3rd full (non-diagonal) triple's exp is offloaded to the DVE as a
     Schraudolph fp16 fast-exp (tensor_scalar mul+add -> round-to-u16 ->
     bitcast fp16, ~3% elementwise, ~3e-3 end-to-end); this trades idle DVE
     time for critical ACT time. attn@[v|1] accumulates per unit into PSUM
     [65,512]; row 64 = softmax denominators.
  C) Normalization: one batched Ln+Exp(-x) reciprocal over the packed
     denominator tiles (2 ACT table switches total), per-head DMA-broadcast
     via a DRAM roundtrip, then y = ao @ Wo_g with PSUM evacuation
     alternating between DVE and ACT.

PSUM (8 banks): "sp" 2x[128,1536] (6) shared by scores triples, phase-C
y_ps, and phase-A q/transpose staging; "o" 2x[65,512] (2) shared by attn@v
accumulators and phase-A kv/k-transpose staging.

The causal mask input is never loaded: exp(-1e9 + s) == 0.0 exactly in fp32.
mm_dt=f16 (default): PE streams 1 cycle/row; rel err ~3.3e-3 incl. fast-exp.
"""
import math
import os
import numpy as np

import concourse.bass as bass
import concourse.mybir as mybir
import concourse.tile as tile
from concourse import bacc
from concourse.masks import make_identity

F32 = mybir.dt.float32
F32R = mybir.dt.float32r
F16 = mybir.dt.float16
U16 = mybir.dt.uint16
AF = mybir.ActivationFunctionType
ALU = mybir.AluOpType
# fp16 Schraudolph fast-exp: bits = round(x*EXPA + EXPB) viewed as fp16.
# |rel err| <= ~3%; applied to a ~1/3 subset of full k-tile groups the
# end-to-end error contribution is ~3e-3 (gate is 2e-2).
EXPA = 1477.3197218702985      # 1024/ln(2)
EXPB = 15293.44                # (15 - 0.0650) * 1024

B, S, HID = 4, 2048, 896
H, KV, D = 14, 2, 64
HL = H // KV          # 7 local query heads per core
GD = HL * D           # 448 local channels
KCH = HID // 128      # 7 contraction chunks
N_CORES = 8


def _bc7(ap_small):
    """[128, 32] cos/sin slice -> broadcast over the 7 heads: [128, 7, 32]."""
    return bass.AP(
        tensor=ap_small.tensor,
        offset=ap_small.offset,
        ap=[list(ap_small.ap[0]), [0, HL], list(ap_small.ap[1])],
    )


def build(s=S, mm_dt=None, reps=1, phases="A2BC"):
    if mm_dt is None:
        mm_dt = MM_DT
    ST = s // 128           # s-tiles
    QC = s // 512           # q chunks
    nc = bacc.Bacc("TRN2", target_bir_lowering=False, debug=False,
                   num_devices=N_CORES)

    xT = nc.dram_tensor("xT", [HID, s], mm_dt, kind="ExternalInput").ap()
    wq = nc.dram_tensor("wq", [HID, GD], mm_dt, kind="ExternalInput").ap()
    wkv = nc.dram_tensor("wkv", [HID, 128], mm_dt, kind="ExternalInput").ap()
    wo = nc.dram_tensor("wo", [GD, HID], mm_dt, kind="ExternalInput").ap()
    cosr = nc.dram_tensor("cosr", [128, ST, 32], F32, kind="ExternalInput").ap()
    sinr = nc.dram_tensor("sinr", [128, ST, 32], F32, kind="ExternalInput").ap()
    triu = nc.dram_tensor("triu", [128, 128], mm_dt, kind="ExternalInput").ap()
    y = nc.dram_tensor("y", [s, HID], F32, kind="ExternalOutput").ap()
    rec_dt0 = F16 if mm_dt == F16 else F32
    dram_rec = nc.dram_tensor("dram_rec", [HL, s], rec_dt0).ap()

    with tile.TileContext(nc) as tc:
        with (
            tc.tile_pool(name="wp", bufs=1) as wp,
            tc.tile_pool(name="per", bufs=1) as per,
            tc.tile_pool(name="tmp", bufs=2) as tmp,
        ):
            def _body():
                # ---- small loads (weights/tables stay resident) ----
                wq_sb = wp.tile([128, KCH, GD], mm_dt, tag="wq", name="wq")
                nc.sync.dma_start(out=wq_sb[:], in_=wq.rearrange("(k p) m -> p k m", p=128))
                wkv_sb = wp.tile([128, KCH, 128], mm_dt, tag="wkv", name="wkv")
                nc.sync.dma_start(out=wkv_sb[:], in_=wkv.rearrange("(k p) m -> p k m", p=128))
                wo_sb = wp.tile([128, 4, HID], mm_dt, tag="wo", name="wo")
                for cc in range(4):
                    w = 128 if cc < 3 else 64
                    nc.sync.dma_start(out=wo_sb[0:w, cc, :], in_=wo[cc * 128:cc * 128 + w, :])
                cos_sb = wp.tile([128, ST, 32], F32, tag="cos", name="cos")
                nc.sync.dma_start(out=cos_sb[:], in_=cosr)
                sin_sb = wp.tile([128, ST, 32], F32, tag="sin", name="sin")
                nc.sync.dma_start(out=sin_sb[:], in_=sinr)
                triu_sb = wp.tile([128, 128], mm_dt, tag="triu", name="triu")
                nc.sync.dma_start(out=triu_sb[:], in_=triu)
                idn = wp.tile([128, 128], F32, tag="idn", name="idn")
                make_identity(nc, idn[:])
                idn_r = wp.tile([128, 128], mm_dt, tag="idnr", name="idnr")
                nc.vector.tensor_copy(idn_r[:], idn[:])
                idn_mm = idn_r[:]
                # PE warmup: ~4us of dummy matmuls while the input DMAs land,
                # so the HAM clock-gate reaches 8/8 before real work starts
                with tc.tile_pool(name="psW", bufs=1, space="PSUM") as psW:
                    wps = psW.tile([128, 128], F32, tag="w", name="w")
                    for _ in range(72):
                        nc.tensor.matmul(wps[:], idn_mm, idn_mm,
                                         start=True, stop=True)

                # ---- persistent intermediates ----
                # qT / aoT unified: [128, 4, s]; chunk j holds heads (2j, 2j+1),
                # head h lives at chunk h//2, partition half (h%2)*64
                q_all = per.tile([128, 4, s], mm_dt, tag="q_all", name="q_all")
                ao_all = per.tile([128, 4, s], mm_dt, tag="ao_all", name="ao_all")
                rec_dt = F16 if mm_dt == F16 else F32
                araw_all = per.tile([128, 4, s], mm_dt, tag="araw", name="araw")
                # denominator rows live at engine-legal base partitions
                # {0,32,64,96}: head h -> tile h//4, row 32*(h%4)
                den_t = [per.tile([128, s], F32, tag=f"dn{j}", name=f"dn{j}")
                         for j in range(2)]
                kT2 = per.tile([128, s], mm_dt, tag="kT2", name="kT2")
                k_all = per.tile([128, ST, 64], mm_dt, tag="k_all", name="k_all")
                v_all = per.tile([128, ST, 65], mm_dt, tag="v_all", name="v_all")
                if mm_dt == F16:
                    nc.vector.memset(
                        v_all[:, :, 64:65].bitcast(mybir.dt.uint16), 0x3C00)
                else:
                    nc.vector.memset(v_all[:, :, 64:65].bitcast(F32), 1.0)

                # ========== pipelined phases: A feeds B; C trails ==========
                # One PSUM pool, 8 banks total:
                #   tag "sp": [128,1536] F32 x2 bufs (6 banks) - scores
                #       triples (GK=3) + phase-C y_ps + A-phase staging
                #   tag "o":  [65,512] F32 x2 (2 banks) - attn@v accumulator
                #       + A-phase kv/transpose staging (pre-B only)
                GK = 3
                LOOK = 1
                with tc.tile_pool(name="xp", bufs=1) as xp, \
                     tc.tile_pool(name="ps", bufs=1, space="PSUM") as ps, \
                     tc.tile_pool(name="expp", bufs=1) as expp:
                    xT_sb = [xp.tile([128, s], mm_dt, tag=f"xT{k}", name=f"xT{k}")
                             for k in range(KCH)]
                    for k in range(KCH):
                        nc.sync.dma_start(out=xT_sb[k][:],
                                          in_=xT[k * 128:(k + 1) * 128, :])

                    # ---- A-chain emitters: 3 steps per s-tile ----
                    rope_pend = {}

                    PSB = {"sp": 2, "o": 2}

                    q_pend = {}

                    def a_st1a(st, tq="aq"):
                        # q projection first half
                        q_ps = ps.tile([128, GD], F32, tag=tq, name=tq,
                                       bufs=PSB[tq])
                        for kc in range(4):
                            nc.tensor.matmul(q_ps[:],
                                             xT_sb[kc][:, st * 128:(st + 1) * 128],
                                             wq_sb[:, kc, :],
                                             start=(kc == 0), stop=False)
                        q_pend[st] = q_ps

                    def a_st1(st, tq="aq"):
                        # q projection second half + RoPE (parked in rope_pend)
                        q_ps = q_pend.pop(st)
                        for kc in range(4, KCH):
                            nc.tensor.matmul(q_ps[:],
                                             xT_sb[kc][:, st * 128:(st + 1) * 128],
                                             wq_sb[:, kc, :],
                                             start=False, stop=(kc == KCH - 1))
                        qv = q_ps[:].rearrange("p (h u two) -> p h u two", two=2, u=32)
                        e, o = qv[:, :, :, 0], qv[:, :, :, 1]
                        cb = _bc7(cos_sb[:, st, :])
                        sb_ = _bc7(sin_sb[:, st, :])
                        t1 = tmp.tile([128, HL, 32], F32, tag="t1", name="t1", bufs=1)
                        t2 = tmp.tile([128, HL, 32], F32, tag="t2", name="t2", bufs=1)
                        t3 = tmp.tile([128, HL, 32], F32, tag="t3", name="t3", bufs=1)
                        t4 = tmp.tile([128, HL, 32], F32, tag="t4", name="t4", bufs=1)
                        nc.vector.tensor_mul(t1[:], e, cb)
                        nc.vector.tensor_mul(t2[:], o, sb_)
                        nc.vector.tensor_mul(t3[:], e, sb_)
                        nc.vector.tensor_mul(t4[:], o, cb)
                        q_rot = tmp.tile([128, GD], mm_dt, tag="qrot", name="qrot",
                                         bufs=2)
                        qrv = q_rot[:].rearrange("p (h u two) -> p h u two", two=2, u=32)
                        nc.gpsimd.tensor_sub(qrv[:, :, :, 0], t1[:], t2[:])
                        nc.gpsimd.tensor_add(qrv[:, :, :, 1], t3[:], t4[:])
                        rope_pend[st] = q_rot

                    def a_st2(st, tkv="aq"):
                        # k/v projection + staging
                        kv_ps = ps.tile([128, 128], F32, tag=tkv, name=tkv,
                                        bufs=PSB[tkv])
                        for kc in range(KCH):
                            nc.tensor.matmul(kv_ps[:],
                                             xT_sb[kc][:, st * 128:(st + 1) * 128],
                                             wkv_sb[:, kc, :],
                                             start=(kc == 0), stop=(kc == KCH - 1))
                        nc.vector.tensor_copy(k_all[:, st, :], kv_ps[:, 0:64])
                        nc.vector.tensor_copy(v_all[:, st, 0:64], kv_ps[:, 64:128])

                    def a_st3(st, ttr="aq"):
                        # PE-transpose q_rot to channel-major q_all
                        q_rot = rope_pend.pop(st)
                        t_ps = ps.tile([128, 4, 128], mm_dt, tag=ttr, name=ttr,
                                       bufs=PSB[ttr])
                        for cc in range(4):
                            w = 128 if cc < 3 else 64
                            nc.tensor.transpose(t_ps[0:w, cc, :],
                                                q_rot[:, cc * 128:cc * 128 + w],
                                                idn_mm)
                        nc.vector.tensor_copy(
                            q_all[:, 0:3, st * 128:(st + 1) * 128], t_ps[:, 0:3, :])
                        nc.vector.tensor_copy(
                            q_all[0:64, 3, st * 128:(st + 1) * 128], t_ps[0:64, 3, :])

                    def a_kstep(c, part=2):
                        # part 0: RoPE on k for chunk c; part 1: transpose+dup
                        if part == 2:
                            a_kstep(c, 0)
                            a_kstep(c, 1)
                            return
                        if part == 1:
                            for st in range(4 * c, 4 * c + 4):
                                t_ps = ps.tile([128, 128], mm_dt, tag="o",
                                               name="o", bufs=2)[0:64, :]
                                nc.tensor.transpose(t_ps, k_all[:, st, :], idn_mm)
                                nc.vector.tensor_copy(
                                    kT2[0:64, st * 128:(st + 1) * 128], t_ps)
                                nc.vector.tensor_copy(
                                    kT2[64:128, st * 128:(st + 1) * 128], t_ps)
                            return
                        kv4 = k_all[:, 4 * c:4 * c + 4, :].rearrange(
                            "p s (u two) -> p s u two", two=2)
                        ke, ko = kv4[:, :, :, 0], kv4[:, :, :, 1]
                        csl = cos_sb[:, 4 * c:4 * c + 4, :]
                        ssl = sin_sb[:, 4 * c:4 * c + 4, :]
                        k1 = tmp.tile([128, 4, 32], F32, tag="t1", name="t1", bufs=1)
                        k2 = tmp.tile([128, 4, 32], F32, tag="t2", name="t2", bufs=1)
                        k3 = tmp.tile([128, 4, 32], F32, tag="t3", name="t3", bufs=1)
                        k4 = tmp.tile([128, 4, 32], F32, tag="t4", name="t4", bufs=1)
                        nc.gpsimd.tensor_mul(k1[:], ke, csl)
                        nc.gpsimd.tensor_mul(k2[:], ko, ssl)
                        nc.gpsimd.tensor_mul(k3[:], ke, ssl)
                        nc.gpsimd.tensor_mul(k4[:], ko, csl)
                        nc.gpsimd.tensor_sub(ke, k1[:], k2[:])
                        nc.gpsimd.tensor_add(ko, k3[:], k4[:])

                    # ---- all of phase A up front (B's ACT stream must
                    # never wait on inserted PE work: PE lead over ACT is
                    # capped at the 2 "sp" slots, so interleaving A into B
                    # stalls the bottleneck engine) ----
                    for c in range(QC):
                        t0, t1, t2, t3 = 4 * c, 4 * c + 1, 4 * c + 2, 4 * c + 3
                        a_st1a(t0, "sp"); a_st1(t0, "sp")
                        a_st1a(t1, "sp"); a_st1(t1, "sp")
                        a_st2(t0, "o"); a_st2(t1, "o")
                        a_st1a(t2, "sp"); a_st1(t2, "sp")
                        a_st1a(t3, "sp"); a_st1(t3, "sp")
                        a_st3(t0, "o"); a_st3(t1, "o")
                        a_st2(t2, "sp"); a_st2(t3, "sp")
                        a_kstep(c, 0)
                        a_st3(t2, "o"); a_st3(t3, "o")
                        a_kstep(c, 1)

                    # ---- B: per-qc flat pipeline over (head, group) ----
                    fullctr = [0]

                    def emit_group(h, qc, gi):
                        half = (h % 2) * 64
                        jq = h // 2
                        nkt = 4 * (qc + 1)
                        s_ps = ps.tile([128, GK * 512], F32, tag="sp", name="sp",
                                       bufs=2)
                        ex = expp.tile([128, GK * 512], mm_dt, tag="ex",
                                       name="ex", bufs=4)
                        info = []
                        for j in range(GK):
                            kt = GK * gi + j
                            if kt >= nkt:
                                break
                            rrel = kt - 4 * qc
                            off = 128 * rrel if rrel >= 0 else 0
                            N = 512 - off
                            nc.tensor.matmul(
                                s_ps[:, 512 * j + off:512 * (j + 1)],
                                kT2[half:half + 64, kt * 128:(kt + 1) * 128],
                                q_all[half:half + 64, jq,
                                      qc * 512 + off:(qc + 1) * 512],
                                start=True, stop=True)
                            info.append((kt, 512 * j + off, off, N, rrel))
                        lo = info[0][1]
                        hi = info[-1][1] + info[-1][3]
                        full = info[-1][4] < 0 and len(info) == GK
                        if full and mm_dt == F16:
                            fullctr[0] += 1
                            if fullctr[0] % 2 == 0:
                                # DVE fast-exp offload: frees the ACT
                                # bottleneck at ~1.6x the DVE cost
                                ft = tmp.tile([128, GK * 512], F32, tag="fet",
                                              name="fet", bufs=2)
                                nc.vector.tensor_scalar(
                                    out=ft[:, lo:hi], in0=s_ps[:, lo:hi],
                                    scalar1=EXPA, scalar2=EXPB,
                                    op0=ALU.mult, op1=ALU.add)
                                nc.vector.tensor_copy(
                                    ex[:, lo:hi].bitcast(U16), ft[:, lo:hi])
                                return ex, info
                        nc.scalar.activation(out=ex[:, lo:hi],
                                             in_=s_ps[:, lo:hi], func=AF.Exp)
                        for kt, base, off, N, rrel in info:
                            if rrel >= 0:
                                nc.vector.tensor_mul(
                                    ex[:, base:base + 128],
                                    ex[:, base:base + 128], triu_sb[:])
                        return ex, info

                    lts = []
                    units = []
                    for qc in range(QC):
                        nkt = 4 * (qc + 1)
                        ngrp = (nkt + GK - 1) // GK
                        for h in range(HL):
                            units.append((h, qc, ngrp, nkt))
                    flat = [(ui, gi) for ui, u in enumerate(units)
                            for gi in range(u[2])]
                    emitted = {}
                    ptr = 0
                    o_ps = {}
                    for idx in range(len(flat)):
                        while ptr < len(flat) and ptr <= idx + LOOK:
                            ui, gi = flat[ptr]
                            h, qc, ngrp, nkt = units[ui]
                            emitted[(ui, gi)] = emit_group(h, qc, gi)
                            ptr += 1
                        ui, gi = flat[idx]
                        h, qc, ngrp, nkt = units[ui]
                        half = (h % 2) * 64
                        jq = h // 2
                        if gi == 0:
                            o_ps[ui] = ps.tile([65, 512], F32, tag="o",
                                               name="o", bufs=2)
                        ex, info = emitted.pop((ui, gi))
                        for kt, base, off, N, rrel in info:
                            nc.tensor.matmul(
                                o_ps[ui][:, off:512], v_all[:, kt, :],
                                ex[:, base:base + N],
                                start=(kt == 0), stop=(kt == nkt - 1))
                        if gi == ngrp - 1:
                            op = o_ps.pop(ui)
                            nc.vector.tensor_copy(
                                araw_all[half:half + 64, jq,
                                         qc * 512:(qc + 1) * 512],
                                op[0:64, :])
                            row = 32 * (h % 4)
                            nc.vector.tensor_copy(
                                den_t[h // 4][row:row + 1,
                                              qc * 512:(qc + 1) * 512],
                                op[64:65, :])

                    # ---- batched softmax normalization (all heads) ----
                    for j in range(2):
                        ltj = tmp.tile([128, s], F32, tag=f"lt{j}",
                                       name=f"lt{j}", bufs=1)
                        nc.scalar.activation(out=ltj[:], in_=den_t[j][:],
                                             func=AF.Ln)
                        lts.append(ltj)
                    rec2 = []
                    for j in range(2):
                        recj = tmp.tile([128, s], rec_dt, tag=f"rc{j}",
                                        name=f"rc{j}", bufs=1)
                        nc.scalar.activation(out=recj[:], in_=lts[j][:],
                                             func=AF.Exp, scale=-1.0)
                        rec2.append(recj)
                    for h in range(HL):
                        row = 32 * (h % 4)
                        nc.sync.dma_start(out=dram_rec[h:h + 1, :],
                                          in_=rec2[h // 4][row:row + 1, :])
                    for h in range(HL):
                        # broadcast via DRAM roundtrip (partition-broadcast
                        # reads are legal on DRAM-source DMAs and pipeline on
                        # the DMA engines)
                        half = (h % 2) * 64
                        rbF = tmp.tile([128, s], rec_dt, tag="rbF", name="rbF")
                        nc.sync.dma_start(out=rbF[:], in_=bass.AP(
                            tensor=dram_rec.tensor, offset=h * s,
                            ap=[[0, 128], [1, s]]))
                        for qc in range(QC):
                            nc.vector.tensor_mul(
                                ao_all[half:half + 64, h // 2,
                                       qc * 512:(qc + 1) * 512],
                                araw_all[half:half + 64, h // 2,
                                         qc * 512:(qc + 1) * 512],
                                rbF[half:half + 64, qc * 512:(qc + 1) * 512])

                    # ---- C: output projection ----
                    for st in range(ST):
                        y_ps = ps.tile([128, HID], F32, tag="sp", name="sp",
                                       bufs=2)
                        for cc in range(4):
                            w = 128 if cc < 3 else 64
                            lhsT = ao_all[0:w, cc, st * 128:(st + 1) * 128]
                            nc.tensor.matmul(y_ps[:, 0:512], lhsT,
                                             wo_sb[0:w, cc, 0:512],
                                             start=(cc == 0), stop=(cc == 3))
                            nc.tensor.matmul(y_ps[:, 512:896], lhsT,
                                             wo_sb[0:w, cc, 512:896],
                                             start=(cc == 0), stop=(cc == 3))
                        y_sb = tmp.tile([128, HID], F32, tag="ysb", name="ysb")
                        if st % 2 == 0:
                            nc.vector.tensor_copy(y_sb[:], y_ps[:, 0:896])
                        else:
                            nc.scalar.copy(out=y_sb[:], in_=y_ps[:, 0:896])
                        nc.sync.dma_start(out=y[st * 128:(st + 1) * 128, :],
                                          in_=y_sb[:])

            if reps > 1:
                with tc.For_i(0, reps, 1):
                    _body()
            else:
                _body()

    nc.compile()
    return nc


# ---------------------------------------------------------------------------
# host-side sharding + execution
# ---------------------------------------------------------------------------

def round_f32r(a):
    """Round fp32 array to fp32r (RNE to 11 mantissa bits) -- bit-exact match
    of the hardware's casting DMA, verified by SBUF readback."""
    b = np.ascontiguousarray(a, dtype=np.float32).view(np.uint32)
    lsb = (b >> np.uint32(12)) & np.uint32(1)
    r = ((b + np.uint32(0x7FF) + lsb) & np.uint32(0xFFFFF000))
    return r.view(np.float32)


MM_DT = {"f32r": F32R, "f16": F16, "f32": F32}[os.environ.get("MM_DT", "f16")]


def _cvt(a, mm_dt):
    if mm_dt == F16:
        return np.ascontiguousarray(np.asarray(a, dtype=np.float32)).astype(np.float16)
    if mm_dt == F32R:
        return round_f32r(a)
    return np.ascontiguousarray(a, dtype=np.float32)


def make_in_maps(x, freqs_cos, freqs_sin, Wq, Wk, Wv, Wo, s=S, mm_dt=None):
    if mm_dt is None:
        mm_dt = MM_DT
    ST = s // 128
    scale = 1.0 / math.sqrt(D)
    cosr = np.ascontiguousarray(
        np.asarray(freqs_cos).reshape(ST, 128, 32).transpose(1, 0, 2)).astype(np.float32)
    sinr = np.ascontiguousarray(
        np.asarray(freqs_sin).reshape(ST, 128, 32).transpose(1, 0, 2)).astype(np.float32)
    triu = _cvt(np.triu(np.ones((128, 128), dtype=np.float32)), mm_dt)
    in_maps = []
    for c in range(N_CORES):
        b, g = c // 2, c % 2
        in_maps.append({
            "xT": _cvt(np.asarray(x)[b].T, mm_dt),
            "wq": _cvt(np.asarray(Wq)[:, g * GD:(g + 1) * GD] * scale, mm_dt),
            "wkv": _cvt(np.concatenate(
                [np.asarray(Wk)[:, g * D:(g + 1) * D],
                 np.asarray(Wv)[:, g * D:(g + 1) * D]], axis=1), mm_dt),
            "wo": _cvt(np.asarray(Wo)[g * GD:(g + 1) * GD, :], mm_dt),
            "cosr": cosr, "sinr": sinr, "triu": triu,
        })
    return in_maps


_RUNNER = None


class _Runner:
    """Minimal SPMD executor over axon PJRT (self-contained copy)."""

    def __init__(self, nc, n_cores):
        import jax
        from jax.sharding import Mesh, PartitionSpec, NamedSharding
        from jax.experimental.shard_map import shard_map
        from concourse.bass2jax import (_bass_exec_p, install_neuronx_cc_hook,
                                        partition_id_tensor)
        install_neuronx_cc_hook()
        self.jax = jax
        self.n_cores = n_cores
        partition_name = (nc.partition_id_tensor.name
                          if nc.partition_id_tensor else None)
        in_names, out_names, out_avals = [], [], []
        for alloc in nc.m.functions[0].allocations:
            if not isinstance(alloc, mybir.MemoryLocationSet):
                continue
            name = alloc.memorylocations[0].name
            if alloc.kind == "ExternalInput":
                if name != partition_name:
                    in_names.append(name)
            elif alloc.kind == "ExternalOutput":
                out_names.append(name)
                out_avals.append(jax.core.ShapedArray(
                    tuple(alloc.tensor_shape), mybir.dt.np(alloc.dtype)))
        self.in_names, self.out_names, self.out_avals = in_names, out_names, out_avals
        n_params, n_outs = len(in_names), len(out_avals)
        all_names = in_names + out_names
        if partition_name is not None:
            all_names.append(partition_name)

        def _body(*args):
            operands = list(args)
            if partition_name is not None:
                operands.append(partition_id_tensor())
            return tuple(_bass_exec_p.bind(
                *operands, out_avals=tuple(out_avals), in_names=tuple(all_names),
                out_names=tuple(out_names), lowering_input_output_aliases=(),
                sim_require_finite=False, sim_require_nnan=False, nc=nc))

        devices = jax.devices()[:n_cores]
        self.mesh = Mesh(np.asarray(devices), ("core",))
        self.sharding = NamedSharding(self.mesh, PartitionSpec("core"))
        in_specs = (PartitionSpec("core"),) * (n_params + n_outs)
        out_specs = (PartitionSpec("core"),) * n_outs
        self.fn = jax.jit(
            shard_map(_body, mesh=self.mesh, in_specs=in_specs,
                      out_specs=out_specs, check_rep=False),
            donate_argnums=tuple(range(n_params, n_params + n_outs)),
            keep_unused=True)
        zshapes = [(n_cores * a.shape[0], *a.shape[1:]) for a in out_avals]
        zdtypes = [a.dtype for a in out_avals]
        self.make_zeros = jax.jit(
            lambda: tuple(jax.numpy.zeros(sh, dt)
                          for sh, dt in zip(zshapes, zdtypes)),
            out_shardings=tuple(self.sharding for _ in zshapes))

    def prep(self, in_maps):
        return [self.jax.device_put(
            np.concatenate([np.asarray(in_maps[c][n]) for c in range(self.n_cores)],
                           axis=0), self.sharding)
            for n in self.in_names]

    def run(self, dev_in):
        return self.fn(*dev_in, *self.make_zeros())

    def split(self, outs):
        res = []
        for c in range(self.n_cores):
            res.append({n: np.asarray(outs[i]).reshape(
                self.n_cores, *self.out_avals[i].shape)[c]
                for i, n in enumerate(self.out_names)})
        return res


def get_runner():
    global _RUNNER
    if _RUNNER is None:
        _RUNNER = _Runner(build(), N_CORES)
    return _RUNNER


def kernel(x, freqs_cos, freqs_sin, mask, Wq, Wk, Wv, Wo):
    x = np.asarray(x, dtype=np.float32)
    in_maps = make_in_maps(np.asarray(x), np.asarray(freqs_cos),
                           np.asarray(freqs_sin), np.asarray(Wq),
                           np.asarray(Wk), np.asarray(Wv), np.asarray(Wo))
    r = get_runner()
    outs = r.run(r.prep(in_maps))
    res = r.split(outs)
    out = np.empty((B, S, HID), dtype=np.float32)
    for b in range(B):
        out[b] = res[2 * b]["y"] + res[2 * b + 1]["y"]
    return out

